# revision 75
# baseline (speedup 1.0000x reference)
"""Trainium2 Bass kernel for Convpass-swintransformer hypernet-mask adapter.

Data-parallel over batch: 8 NeuronCores x 8 samples each, all weights
replicated. All matmuls run in bf16 (fp32 PSUM accumulation); samples are
processed in pairs with block-diagonal weights so the 64-channel ops fill the
128-wide PE array. Biases are folded into activation/vector drains (or, for
the hypernet, a ones-row on the feat tile) instead of PE bias matmuls.
QuickGELU is a single Gelu_apprx_sigmoid activation; phase-B gelu ops are
gated behind phase A so the ACT function-table isn't thrashed against Exp.
"""
import sys

sys.path.insert(0, "/opt/trn_rl_repo")

import numpy as np

import concourse.bass as bass
import concourse.tile as tile
from concourse import bacc, mybir
from concourse.bass_utils import run_bass_kernel_spmd

AF = mybir.ActivationFunctionType
ALU = mybir.AluOpType
FP32 = mybir.dt.float32
BF16 = mybir.dt.bfloat16
BF16_NP = mybir.dt.np(BF16)

B, L, C = 64, 784, 384
DIM, NM, META = 64, 16, 64
HH, WW = 28, 28
NCORES = 8
S = B // NCORES          # samples per core
NPAIR = S // 2
KC = C // 128            # 3 contraction chunks for C=384
NPOS = [(0, 512), (512, 272)]   # 784 split at psum-bank boundary
NPOSB = [(0, 448), (448, 336)]  # 784 split matching the conv-psum halves
GROUPS = [(0, 3), (3, 3), (6, 2)]   # (first j8, n slots) per hypernet group

_CACHE = {}


def _build_nc():
    nc = bacc.Bacc(None)
    d = nc.declare_dram_parameter
    xt_d = d("xt", [NPAIR, 2, KC, 128, L], BF16, isOutput=False)
    cb_d = d("cb", [128, 2 * 384 + 64 + 32 + 128 + 384], BF16, isOutput=False)
    fb_d = d("fb", [128, 5], FP32, isOutput=False)
    ub_d = d("ub", [128, 3], FP32, isOutput=False)
    hw_d = d("hw", [65, 2, 9, 2048], BF16, isOutput=False)
    out_d = d("out", [S, KC, 128, L], BF16, isOutput=True)

    with tile.TileContext(nc) as tc:
        with tc.tile_pool(name="consts", bufs=1) as cp, \
             tc.tile_pool(name="hxp", bufs=2 * NPAIR) as hxp, \
             tc.tile_pool(name="padp", bufs=NPAIR) as padp, \
             tc.tile_pool(name="cwp", bufs=NPAIR) as cwpp, \
             tc.tile_pool(name="cwall", bufs=1) as cwp, \
             tc.tile_pool(name="hwp", bufs=9) as hwp:
            # ---- constants (two blobs) ----
            cb = cp.tile([128, 2 * 384 + 64 + 32 + 128 + 384], BF16)
            fb = cp.tile([128, 5], FP32)
            upb3 = cp.tile([128, 3], FP32)
            wab = [cb[:, 0:384].rearrange("p (k m) -> p k m", k=KC),
                   cb[:, 384:768].rearrange("p (k m) -> p k m", k=KC)]
            mw2dup = cb[:, 768:832]          # meta_w2 dup'd on both halves
            mtT2 = cb[:, 832:864]            # [128 n-pair, 32 m-pair]
            ones32 = cb[0:32, 864:992]
            upw = cb[:, 992:1376]            # upw dup'd on both halves
            b_a = [fb[:, 0:1], fb[:, 1:2]]   # per-sample-parity psa bias
            mb2p = fb[:, 2:3]
            bnd = [fb[:, 3:4], fb[:, 4:5]]   # relu bound: 0 on h-rows,
                                             # -3e38 on xd-rows, per parity

            feat_bf = cp.tile([65, 32], BF16)
            nc.vector.memset(feat_bf[:], 0.0)
            nc.vector.memset(feat_bf[64:65, :], 1.0)
            warm = cp.tile([64, 512], BF16)
            nc.vector.memset(warm[:], 0.0)
            with tc.tile_pool(name="psW", bufs=1, space="PSUM") as psW:
                psw = psW.tile([64, 512], FP32)
                for _ in range(3):
                    nc.tensor.matmul(psw[:], lhsT=warm[:, 0:64], rhs=warm[:],
                                     start=True, stop=True)
            featP = cp.tile([128, NPAIR], FP32)
            featQ = cp.tile([128, NPAIR], FP32)
            featPQ = [featP, featQ]
            zgate = cp.tile([128, 1], FP32)

            hx_tiles, pad_tiles, cw_tiles = [], [], []
            for s in range(S):
                hx = hxp.tile([128, L], BF16, name=f"hx{s}", tag="hx")
                hx_tiles.append(hx)
            for pr in range(NPAIR):
                pad = padp.tile([128, 900], BF16)
                nc.gpsimd.memset(pad[:], 0.0)
                pad_tiles.append(pad)
                cw = cwpp.tile([128, 9, 2, 64], BF16)
                nc.gpsimd.memset(cw[:], 0.0)
                cw_tiles.append(cw)

            # ================= phase A: meta-net / masks / feat =============
            # Software-pipelined: pair p+1's psa matmuls are interleaved
            # between pair p's small matmuls so the PE never drains, and the
            # ACT/DVE queues are emitted critical-path-first.
            with tc.tile_pool(name="xtp", bufs=4) as xtp, \
                 tc.tile_pool(name="psA", bufs=2, space="PSUM") as psA, \
                 tc.tile_pool(name="psB", bufs=4, space="PSUM") as psB, \
                 tc.tile_pool(name="sbA", bufs=4) as sbA, \
                 tc.tile_pool(name="smallA", bufs=4) as smA:
                xts = [None] * NPAIR
                prompts = [None] * NPAIR

                def emit_xt_dma(pr):
                    xt = xtp.tile([128, 2, KC, L], BF16, name=f"xt{pr}", tag="xt")
                    xts[pr] = xt
                    if pr == 0:
                        nc.sync.dma_start(   # first compute chunk leads
                            out=xt[:, 0, 0, :],
                            in_=xt_d[0, 0, 0].rearrange("p q -> p q"))
                        nc.sync.dma_start(out=cb[:], in_=cb_d[:])
                        for k in range(1, KC):
                            nc.sync.dma_start(
                                out=xt[:, 0, k, :],
                                in_=xt_d[0, 0, k].rearrange("p q -> p q"))
                        nc.sync.dma_start(
                            out=xt[:, 1, :, :],
                            in_=xt_d[0, 1].rearrange("k p q -> p k q"))
                        nc.sync.dma_start(out=fb[:], in_=fb_d[:])
                        nc.sync.dma_start(out=upb3[:], in_=ub_d[:])
                    else:
                        nc.sync.dma_start(
                            out=xt[:], in_=xt_d[pr].rearrange(
                                "s k p q -> p s k q"))

                def emit_psa(pr, h):
                    psa = psA.tile([128, L], FP32, tag="psa")
                    for n0, nw in NPOS:
                        for k in range(KC):
                            nc.tensor.matmul(
                                psa[:, n0:n0 + nw], lhsT=wab[h][:, k, :],
                                rhs=xts[pr][:, h, k, n0:n0 + nw],
                                start=(k == 0), stop=(k == KC - 1))
                    return psa

                def emit_hx(pr, h, psa):
                    # one fused drain per sample: (psa + bias) max bound,
                    # where bound is 0 on the h-rows (= relu) and -3e38 on
                    # the xd-rows (= identity). h-part at rows 64h, xd-part
                    # at rows 64*(1-h); phase B flips h to match.
                    nc.vector.tensor_scalar(
                        hx_tiles[2 * pr + h][:], psa[:],
                        b_a[h], bnd[h], ALU.add, ALU.max)

                # prologue: pair 0 psa + drains
                emit_xt_dma(0)
                for h in range(2):
                    psa = emit_psa(0, h)
                    emit_hx(0, h, psa)

                # All psB products split into 392-wide chunks: each chunk is
                # exactly one PSUM bank, so pool slots recycle at chunk
                # granularity and the psp(p+1) <- exp(p) coupling vanishes.
                CH = [(0, 512), (512, 272)]
                for pr in range(NPAIR):
                    nxt = pr + 1 if pr + 1 < NPAIR else None
                    prompt = sbA.tile([128, L], BF16, tag="prompt")
                    prompts[pr] = prompt
                    for n0, nw in CH:
                        psp = psB.tile([128, nw], FP32, tag="psb")
                        for h in range(2):
                            q0 = 64 * h     # h-part rows of sample 2pr+h
                            nc.tensor.matmul(
                                psp[q0:q0 + 64, :],
                                lhsT=mw2dup[q0:q0 + 64, :],
                                rhs=hx_tiles[2 * pr + h][q0:q0 + 64,
                                                         n0:n0 + nw],
                                start=True, stop=True)
                        nc.scalar.activation(prompt[:, n0:n0 + nw],
                                             psp[:], AF.Identity,
                                             bias=mb2p)

                    psa_n = [None, None]
                    if nxt is not None:
                        emit_xt_dma(nxt)
                        psa_n[0] = emit_psa(nxt, 0)

                    expt = sbA.tile([32, L], BF16, tag="expt")
                    zsum = [None, None]
                    for c, (n0, nw) in enumerate(CH):
                        psm = psB.tile([32, nw], FP32, tag="psb")
                        nc.tensor.matmul(psm[:], lhsT=mtT2,
                                         rhs=prompt[:, n0:n0 + nw],
                                         start=True, stop=True)
                        zsum[c] = smA.tile([32, 1], FP32, tag=f"z{c}",
                                           name=f"zs{pr}_{c}")
                        nc.scalar.activation(expt[:, n0:n0 + nw], psm[:],
                                             AF.Exp, accum_out=zsum[c][:])
                    invz = smA.tile([32, 1], FP32, tag="iz")
                    nc.vector.tensor_tensor(zsum[0][:], zsum[0][:],
                                            zsum[1][:], ALU.add)
                    nc.vector.reciprocal(invz[:], zsum[0][:])

                    if nxt is not None:
                        psa_n[1] = emit_psa(nxt, 1)

                    wones = sbA.tile([32, 128], BF16, tag="wones")
                    nc.vector.tensor_scalar_mul(wones[:], ones32, invz[:])
                    ftmp = sbA.tile([128, L], BF16, tag="ftmp")
                    for c, (n0, nw) in enumerate(CH):
                        pss = psB.tile([128, nw], FP32, tag="psb")
                        nc.tensor.matmul(pss[:], lhsT=wones[:],
                                         rhs=expt[:, n0:n0 + nw],
                                         start=True, stop=True)
                        nc.vector.tensor_mul(ftmp[:, n0:n0 + nw],
                                             pss[:],
                                             prompts[pr][:, n0:n0 + nw])
                        junk = sbA.tile([128, 392], BF16, tag="junk")
                        nc.scalar.activation(junk[:], ftmp[:, n0:n0 + nw],
                                             AF.Identity,
                                             accum_out=featPQ[c][:, pr:pr + 1])

                    if nxt is not None:
                        for h in range(2):
                            emit_hx(nxt, h, psa_n[h])

                    nc.vector.tensor_tensor(featP[:, pr:pr + 1],
                                            featP[:, pr:pr + 1],
                                            featQ[:, pr:pr + 1], ALU.add)
                    nc.vector.tensor_copy(feat_bf[0:64, 2 * pr:2 * pr + 1],
                                          featP[0:64, pr:pr + 1])
                    nc.vector.tensor_copy(feat_bf[0:64, 2 * pr + 1:2 * pr + 2],
                                          featP[64:128, pr:pr + 1])
                    if pr == NPAIR - 1:
                        # zero "gate": carries a dep on the last phase-A op so
                        # the scheduler can't hoist phase-B gelu (and its act
                        # table load) into phase A
                        nc.vector.tensor_scalar(
                            zgate[:], ftmp[:, 0:1], 0.0, None, ALU.mult)

            # ================= phase H: hypernet conv weights ===============
            # full-partition tiles: the strided-partition scatter reads then
            # stay inside one allocation for the access tracker
            cwalls = [cwp.tile([128, 9 * 512], BF16,
                               name=f"cwall{g}", tag=f"cwall{g}")
                      for g, (_, nb) in enumerate(GROUPS)]
            # pad-gelu for every pair only needs xd + the phase-A gate: emit
            # here so ACT runs them (and the gelu table load) during phase H
            for pr in range(NPAIR):
                pad3s = pad_tiles[pr].rearrange("p (r c) -> p r c", r=30)
                for h in range(2):
                    q0 = 64 - 64 * h    # xd rows of sample 2pr+h
                    nc.scalar.activation(
                        pad3s[q0:q0 + 64, 1:29, 1:29],
                        hx_tiles[2 * pr + h][q0:q0 + 64, :].rearrange(
                            "p (a b) -> p a b", a=28)[:],
                        AF.Gelu_apprx_sigmoid, bias=zgate[q0:q0 + 64])
            with tc.tile_pool(name="psH", bufs=6, space="PSUM") as psH:
                rot = 0
                for n9 in range(9):
                    hwc = hwp.tile([65, 2, 2048], BF16, tag="hw")
                    # Pool SWDGE queue: keeps these 9 transfers off the SP
                    # sequencer (~1.2us each there) and off the shared HWDGE
                    nc.gpsimd.dma_start(out=hwc[:], in_=hw_d[:, :, n9, :])
                    for g, (j8_0, nb) in enumerate(GROUPS):
                        psh = psH.tile([32 * nb, 512], FP32, tag="psh")
                        for a in range(nb):
                            h2, j4 = divmod(j8_0 + a, 4)
                            nc.tensor.matmul(
                                psh[32 * a:32 * a + 32, :],
                                lhsT=feat_bf[:, :],
                                rhs=hwc[:, h2, j4 * 512:(j4 + 1) * 512],
                                start=True, stop=True)
                        dst = cwalls[g][0:32 * nb, n9 * 512:(n9 + 1) * 512]
                        if rot == 0:
                            nc.vector.tensor_copy(dst, psh[:])
                        else:
                            nc.scalar.activation(dst, psh[:], AF.Copy)
                        rot = (rot + 1) % 2

            # pair-major, group-ascending inside a pair: the six scatters the
            # first conv needs come first, ordered to match drain completion.
            # Pairs 0-1 on the SP queue, pairs 2-3 on Pool SWDGE, so neither
            # queue's ~1.1-1.2us/DMA dispatch delays the first convs or the
            # output DMAs queued behind them.
            for pr in range(NPAIR):
                for g, (j8_0, nb) in enumerate(GROUPS):
                    for h in range(2):
                        s = 2 * pr + h
                        q = 1 - h       # storage half (matches xd layout)
                        eng = nc.sync if h == 0 else nc.gpsimd
                        eng.dma_start(
                            out=cw_tiles[pr][64 * q + 8 * j8_0:
                                             64 * q + 8 * (j8_0 + nb), :, q, :],
                            in_=cwalls[g][s:32 * nb:32].rearrange(
                                "p (i k o) -> p i k o", i=8, k=9))

            # ================= phase B: adapter conv + up ===================
            with tc.tile_pool(name="yap", bufs=3) as yap, \
                 tc.tile_pool(name="outp", bufs=4) as outp, \
                 tc.tile_pool(name="psC0", bufs=2, space="PSUM") as psC0, \
                 tc.tile_pool(name="psC1", bufs=2, space="PSUM") as psC1, \
                 tc.tile_pool(name="psU", bufs=2, space="PSUM") as psU:
                rot = 0
                convp = [None] * NPAIR

                def emit_conv(pr, taps):
                    pad3 = pad_tiles[pr].rearrange("p (r c) -> p r c", r=30)
                    if convp[pr] is None:
                        convp[pr] = (psC0.tile([128, 448], FP32, name=f"c0_{pr}", tag="c0"),
                                     psC1.tile([128, 336], FP32, name=f"c1_{pr}", tag="c1"))
                    ps0, ps1 = convp[pr]
                    for k9 in taps:
                        ky, kx = divmod(k9, 3)
                        lw = cw_tiles[pr][:, k9, :, :]
                        nc.tensor.matmul(
                            ps0[:], lhsT=lw,
                            rhs=pad3[:, ky:ky + 16, kx:kx + 28],
                            start=(k9 == 0), stop=(k9 == 8))
                        nc.tensor.matmul(
                            ps1[:], lhsT=lw,
                            rhs=pad3[:, ky + 16:ky + 28, kx:kx + 28],
                            start=(k9 == 0), stop=(k9 == 8))

                def emit_up(pr):
                    nonlocal rot
                    ps0, ps1 = convp[pr]
                    ya = yap.tile([128, L], BF16, tag="ya")
                    nc.scalar.activation(ya[:, 0:448], ps0[:],
                                         AF.Gelu_apprx_sigmoid)
                    nc.scalar.activation(ya[:, 448:784], ps1[:],
                                         AF.Gelu_apprx_sigmoid)
                    last = pr == NPAIR - 1
                    for q in range(2):
                        h = 1 - q       # sample parity stored in half q
                        outt = outp.tile([128, KC, L], BF16, tag="outt")
                        for j3 in range(KC):
                            # one 2-bank psum tile per (h, j3); the two
                            # matmuls split at the bank boundary, one drain
                            psu = psU.tile([128, L], FP32, tag="psu")
                            for n0, nw in NPOS:
                                nc.tensor.matmul(
                                    psu[:, n0:n0 + nw],
                                    lhsT=upw[64 * q:64 * q + 64,
                                             128 * j3:128 * (j3 + 1)],
                                    rhs=ya[64 * q:64 * q + 64, n0:n0 + nw],
                                    start=True, stop=True)
                            dst = outt[:, j3, :]
                            on_dve = (rot % 2 == 0) if last else (rot % 3 != 2)
                            if on_dve:
                                nc.vector.tensor_scalar(
                                    dst, psu[:], upb3[:, j3:j3 + 1],
                                    None, ALU.add)
                            else:
                                nc.scalar.activation(
                                    dst, psu[:], AF.Identity,
                                    bias=upb3[:, j3:j3 + 1])
                            rot += 1
                            if last and q == 0:
                                # chunked final DMA shortens the drain tail
                                nc.sync.dma_start(
                                    out=out_d[2 * pr + h, j3].rearrange(
                                        "p q -> p q"),
                                    in_=outt[:, j3, :])
                        if not (last and q == 0):
                            nc.sync.dma_start(
                                out=out_d[2 * pr + h].rearrange(
                                    "k p q -> p k q"),
                                in_=outt[:])

                # software pipeline: up(p)'s qgelu latency is covered by the
                # first taps of conv(p+1); its drain tail by the rest
                emit_conv(0, range(9))
                for pr in range(1, NPAIR):
                    emit_conv(pr, range(0, 5))
                    emit_up(pr - 1)
                    emit_conv(pr, range(5, 9))
                emit_up(NPAIR - 1)
    nc.finalize()
    return nc


def _prep(x, meta_w1, meta_b1, meta_w2, meta_b2, mask_token,
          hyper_w, hyper_b, down_w, down_b, up_w, up_b):
    f = lambda a: np.ascontiguousarray(np.asarray(a, dtype=np.float32))
    bf = lambda a: np.ascontiguousarray(np.asarray(a).astype(BF16_NP))
    x = f(x)
    xt = x.reshape(B, L, C).transpose(0, 2, 1)            # [B, C, L]
    xt = bf(xt).reshape(B, KC, 128, L).reshape(NCORES, NPAIR, 2, KC, 128, L)

    # psa weights: even parity [meta_w1 | down_w], odd parity swapped
    wA0 = np.concatenate([f(meta_w1), f(down_w)], axis=1)   # [384, 128]
    wA1 = np.concatenate([f(down_w), f(meta_w1)], axis=1)
    wab = np.stack([wA0, wA1]).reshape(2, KC, 128, 128).transpose(
        0, 2, 1, 3).reshape(2, 128, 384)                    # [par, p, (k m)]

    mtT2s = np.zeros((32, 128), np.float32)     # [m-pair, n-pair]
    mtT2s[0:16, 0:64] = f(mask_token)
    mtT2s[16:32, 64:128] = f(mask_token)
    ones32 = np.zeros((32, 128), np.float32)
    ones32[0:16, 0:64] = 1.0
    ones32[16:32, 64:128] = 1.0
    upw = f(up_w)                                # [64, 384]

    cbw = 2 * 384 + 64 + 32 + 128 + 384
    cb = np.zeros((128, cbw), np.float32)
    cb[:, 0:384] = wab[0]
    cb[:, 384:768] = wab[1]
    cb[0:64, 768:832] = f(meta_w2)
    cb[64:128, 768:832] = f(meta_w2)            # dup for base-partition match
    cb[:, 832:864] = mtT2s.T                    # [128 n-pair, 32 m-pair]
    cb[0:32, 864:992] = ones32
    cb[0:64, 992:1376] = upw
    cb[64:128, 992:1376] = upw                  # dup for base-partition match
    cb = bf(cb)

    fbm = np.zeros((128, 5), np.float32)
    fbm[0:64, 0] = f(meta_b1)
    fbm[64:128, 0] = f(down_b)
    fbm[0:64, 1] = f(down_b)
    fbm[64:128, 1] = f(meta_b1)
    fbm[0:64, 2] = f(meta_b2)
    fbm[64:128, 2] = f(meta_b2)
    fbm[0:64, 3] = 0.0                          # relu bound, parity 0
    fbm[64:128, 3] = -3.0e38
    fbm[0:64, 4] = -3.0e38                      # relu bound, parity 1
    fbm[64:128, 4] = 0.0

    upb3 = f(up_b).reshape(KC, 128).T            # [128, 3]

    # hypernet weights: columns packed (j8, i8, ky, kx, o); ones-row = hyper_b
    hw5 = f(hyper_w).reshape(META, DIM, DIM, 3, 3)       # [n, o, i, ky, kx]
    hwc = hw5.transpose(0, 2, 3, 4, 1).reshape(META, 8, 4608)  # [n, j8, (i8 k o)]
    hwc = hwc.reshape(META, 2, 4, 9, 512).transpose(0, 1, 3, 2, 4).reshape(
        META, 2, 9, 2048)
    hb5 = f(hyper_b).reshape(DIM, DIM, 3, 3)             # [o, i, ky, kx]
    hbc = hb5.transpose(1, 2, 3, 0).reshape(8, 4608)     # [j8, (i8 k o)]
    hbc = hbc.reshape(2, 4, 9, 512).transpose(0, 2, 1, 3).reshape(2, 9, 2048)
    hwe = np.concatenate([hwc, hbc[None]], axis=0)       # [65, 2, 9, 2048]
    hwe = bf(hwe)

    consts = {"cb": cb, "fb": fbm, "hw": hwe, "ub": np.ascontiguousarray(upb3)}
    in_maps = []
    for c in range(NCORES):
        m = dict(consts)
        m["xt"] = np.ascontiguousarray(xt[c])
        in_maps.append(m)
    return in_maps


def _run(in_maps, **kw):
    if "nc" not in _CACHE:
        _CACHE["nc"] = _build_nc()
    return run_bass_kernel_spmd(_CACHE["nc"], in_maps, list(range(NCORES)), **kw)


def kernel(x, meta_w1, meta_b1, meta_w2, meta_b2, mask_token,
           hyper_w, hyper_b, down_w, down_b, up_w, up_b, H, W):
    assert int(H) == HH and int(W) == WW
    in_maps = _prep(x, meta_w1, meta_b1, meta_w2, meta_b2, mask_token,
                    hyper_w, hyper_b, down_w, down_b, up_w, up_b)
    res = _run(in_maps)
    outs = []
    for c in range(NCORES):
        o = np.asarray(res.results[c]["out"]).astype(np.float32)
        # [S, KC, 128, L] -> [S, C, L] -> [S, L, C]
        o = o.reshape(S, C, L).transpose(0, 2, 1)
        outs.append(o)
    out = np.concatenate(outs, axis=0)
    return np.ascontiguousarray(out.reshape(B, L, C)).astype(np.float32)
